# revision 1
# baseline (speedup 1.0000x reference)
"""Trainium2 Bass kernel for nn_Attention_3934190044008.

Multi-head attention with additive bias and sigmoid gating:
  q = (q_x @ w_q) / 8, k = kv_x @ w_k, v = kv_x @ w_v   (8 heads x 64)
  a = softmax(q k^T + bias);  o = a @ v
  o = o * sigmoid(q_x @ w_g + b_g);  out = o @ w_o + b_o

Sharding: 16 (batch, head) pairs over 8 cores -> each core owns one batch
element and 2 heads, produces a partial [2048, 256] output contribution
(o_slice @ w_o rows); host sums the 4 partials per batch and adds b_o.

Device-side layout is "feature on partitions" (transposed): scores are
computed as S^T [k, q] so the softmax denominator rides the AV matmul via a
ones-column appended to V, and softmax-over-k never needs a partition-axis
reduction. All transposes are done on the host (numpy) when building the
per-core input maps.
"""

import os
import sys
import threading
from contextlib import ExitStack

import numpy as np

_REPO = "/opt/trn_rl_repo"
if _REPO not in sys.path and os.path.isdir(_REPO):
    sys.path.insert(0, _REPO)

import concourse.bass as bass  # noqa: E402
import concourse.mybir as mybir  # noqa: E402
import concourse.tile as tile  # noqa: E402
from concourse import bacc  # noqa: E402
from concourse.bass_utils import run_bass_kernel_spmd  # noqa: E402

F32 = mybir.dt.float32
F32R = mybir.dt.float32r

B, SEQ, CQ = 2, 2048, 256
H, DH = 8, 64
HD = H * DH  # 512
N_CORES = 8
HPC = 2  # heads per core

# dtype knobs:
#   QK: "bf16split" (hi/lo bf16 3-product, ~1e-5 on scores), "f32r" (tf32,
#       ~1e-3 on scores), or "f32" (exact, but 4 cycles/row on the PE)
#   AV: "f32r" (tf32) or "f32"
QK_DT = os.environ.get("KRN_QK_DT", "bf16split")
AV_DT = os.environ.get("KRN_AV_DT", "f32r")


def _dt(kind):
    return F32R if kind == "f32r" else F32


def build_nc():
    nc = bacc.Bacc("TRN2", target_bir_lowering=False, debug=False)

    qxT = nc.dram_tensor("qxT", [CQ, SEQ], F32, kind="ExternalInput").ap()
    kvxT = nc.dram_tensor("kvxT", [CQ, SEQ], F32, kind="ExternalInput").ap()
    biasT = nc.dram_tensor("biasT", [HPC, SEQ, SEQ], F32, kind="ExternalInput").ap()
    wq = nc.dram_tensor("wq", [CQ, HPC * DH], F32, kind="ExternalInput").ap()
    wk = nc.dram_tensor("wk", [CQ, HPC * DH], F32, kind="ExternalInput").ap()
    wv = nc.dram_tensor("wv", [CQ, HPC * DH], F32, kind="ExternalInput").ap()
    wg = nc.dram_tensor("wg", [CQ, HPC * DH], F32, kind="ExternalInput").ap()
    bg = nc.dram_tensor("bg", [HPC * DH, 1], F32, kind="ExternalInput").ap()
    wo = nc.dram_tensor("wo", [HPC * DH, CQ], F32, kind="ExternalInput").ap()
    # per-head unnormalized partials + softmax denominators; the division
    # and cross-core summation happen on the host after the gather
    outs_d = [nc.dram_tensor(f"out{h}", [SEQ, CQ], F32, kind="ExternalOutput").ap()
              for h in range(HPC)]
    rs_d = nc.dram_tensor("rs", [1, HPC, SEQ], F32, kind="ExternalOutput").ap()

    NKT = SEQ // 128  # 16 k-tiles
    P = 128

    with tile.TileContext(nc) as tc:
        with ExitStack() as ctx:
            singles = ctx.enter_context(tc.tile_pool(name="singles", bufs=1))

            # ---- resident SBUF tensors ----
            # weights first (tiny, they gate the first projection matmuls);
            # one strided DMA per weight to minimize ~600ns-per-issue
            # sequencer serialization at startup
            w_sbs = {}
            for name, src in (("wk", wk), ("wq", wq), ("wv", wv), ("wg", wg)):
                t = singles.tile([P, 2, P], F32, tag=f"w_{name}")
                eng = nc.sync if name in ("wk", "wv") else nc.scalar
                eng.dma_start(t, src.rearrange("(a p) c -> p a c", p=P))
                w_sbs[name] = t
            bg_sb = singles.tile([P, 1], F32)
            nc.sync.dma_start(bg_sb, bg)
            wo_sb = singles.tile([DH, HPC, CQ], F32)
            nc.scalar.dma_start(wo_sb, wo.rearrange("(h p) c -> p h c", p=DH))

            # inputs as 1 MB halves, K-path first (it gates the first matmuls)
            qxT_sb = singles.tile([P, 2, SEQ], F32)
            kvxT_sb = singles.tile([P, 2, SEQ], F32)
            for a in range(2):
                (nc.sync if a == 0 else nc.scalar).dma_start(
                    kvxT_sb[:, a, :], kvxT[a * P:(a + 1) * P, :])
            for a in range(2):
                (nc.sync if a == 0 else nc.scalar).dma_start(
                    qxT_sb[:, a, :], qxT[a * P:(a + 1) * P, :])

            BF16 = mybir.dt.bfloat16
            if QK_DT == "bf16split":
                # hi/lo bf16 decomposition: S = Kh.Qh + Kl.Qh + Kh.Ql
                # (drops Kl.Ql, ~2^-18 relative on scores).
                KhKl = [singles.tile([P, SEQ], BF16, name=f"KhKl{h}", tag=f"khkl{h}")
                        for h in range(HPC)]  # rows 0-63 Kh, 64-127 Kl
                QhQh = [singles.tile([P, SEQ], BF16, name=f"QhQh{h}", tag=f"qhqh{h}")
                        for h in range(HPC)]  # Qh duplicated on both halves
                Qlo = [singles.tile([DH, SEQ], BF16, name=f"Qlo{h}", tag=f"qlo{h}")
                       for h in range(HPC)]
                KT_sb = QT_sb = None
            else:
                KT_sb = singles.tile([P, SEQ], _dt(QK_DT))   # [2h x 64 d, k]
                QT_sb = singles.tile([P, SEQ], _dt(QK_DT))   # [2h x 64 d, q]
            GT_sb = singles.tile([P, SEQ], F32)  # gate, [2 heads x 64, q]
            V_sb = singles.tile([P, HPC, NKT, DH + 1], _dt(AV_DT))  # [k%128, h, kt, d|1]
            OG_sb = singles.tile([DH, HPC, SEQ], F32)  # (o * g)^T, final lhsT
            rs_sb = singles.tile([1, HPC, SEQ], F32)   # softmax denominators
            ones_col = V_sb[:, :, :, DH:DH + 1]
            if ones_col.dtype == F32R:
                ones_col = ones_col.bitcast(F32)
            nc.vector.memset(ones_col, 1.0)

            # ---- stage B: projections ----
            with tc.tile_pool(name="ppsum", bufs=2, space="PSUM") as ppool, \
                 tc.tile_pool(name="klop", bufs=2) as klop:
                if QK_DT == "bf16split":
                    # Kl staging lives only until the dup-DMA copies it into
                    # KhKl rows 64-127; scoped pool frees its SBUF for the
                    # attention-phase pools
                    Klo_t = [klop.tile([DH, SEQ], BF16, name=f"Klo{h}",
                                       tag="klo") for h in range(HPC)]
                    # per-head M=64 projections so hi/lo tiles land on
                    # partitions 0-63 (DVE cannot move data across partitions)
                    for h in range(HPC):
                        hc = slice(h * DH, (h + 1) * DH)
                        for wt, x_sb, hi, hirow, lo in (
                                (w_sbs["wk"], kvxT_sb, KhKl[h], 0, Klo_t[h]),
                                (w_sbs["wq"], qxT_sb, QhQh[h], 0, Qlo[h])):
                            for tt in range(SEQ // 512):
                                ps = ppool.tile([DH, 512], F32, tag="proj64")
                                nc.tensor.matmul(ps, wt[:, 0, hc],
                                                 x_sb[:, 0, bass.ts(tt, 512)],
                                                 start=True, stop=False)
                                nc.tensor.matmul(ps, wt[:, 1, hc],
                                                 x_sb[:, 1, bass.ts(tt, 512)],
                                                 start=False, stop=True)
                                nc.scalar.copy(hi[0:DH, bass.ts(tt, 512)], ps)
                                nc.vector.tensor_sub(lo[:, bass.ts(tt, 512)], ps,
                                                     hi[0:DH, bass.ts(tt, 512)])
                        # duplicate Qh to rows 64-127; move Kl there too
                        nc.sync.dma_start(QhQh[h][DH:P, :], QhQh[h][0:DH, :])
                        nc.sync.dma_start(KhKl[h][DH:P, :], Klo_t[h][:, :])
                else:
                    for wt, x_sb, dst in ((w_sbs["wq"], qxT_sb, QT_sb),
                                          (w_sbs["wk"], kvxT_sb, KT_sb)):
                        for tt in range(SEQ // 512):
                            ps = ppool.tile([P, 512], F32, tag="proj")
                            nc.tensor.matmul(ps, wt[:, 0, :],
                                             x_sb[:, 0, bass.ts(tt, 512)],
                                             start=True, stop=False)
                            nc.tensor.matmul(ps, wt[:, 1, :],
                                             x_sb[:, 1, bass.ts(tt, 512)],
                                             start=False, stop=True)
                            nc.vector.tensor_copy(dst[:, bass.ts(tt, 512)], ps)
                # gate projection + sigmoid (+ b_g as per-partition bias)
                for tt in range(SEQ // 512):
                    ps = ppool.tile([P, 512], F32, tag="projg")
                    nc.tensor.matmul(ps, w_sbs["wg"][:, 0, :],
                                     qxT_sb[:, 0, bass.ts(tt, 512)],
                                     start=True, stop=False)
                    nc.tensor.matmul(ps, w_sbs["wg"][:, 1, :],
                                     qxT_sb[:, 1, bass.ts(tt, 512)],
                                     start=False, stop=True)
                    nc.scalar.activation(GT_sb[:, bass.ts(tt, 512)], ps,
                                         mybir.ActivationFunctionType.Sigmoid,
                                         bias=bg_sb)
                # V projection: out rows = tokens(k), cols = 2 heads x 64
                for kt in range(NKT):
                    ps = ppool.tile([P, P], F32, tag="vproj")
                    nc.tensor.matmul(ps, kvxT_sb[:, 0, bass.ts(kt, P)], w_sbs["wv"][:, 0, :],
                                     start=True, stop=False)
                    nc.tensor.matmul(ps, kvxT_sb[:, 1, bass.ts(kt, P)], w_sbs["wv"][:, 1, :],
                                     start=False, stop=True)
                    nc.vector.tensor_copy(V_sb[:, 0, kt, 0:DH], ps[:, 0:DH])
                    nc.vector.tensor_copy(V_sb[:, 1, kt, 0:DH], ps[:, DH:2 * DH])

            # ---- stage C: attention ----
            # kt-outer / q-block-inner: one contiguous 1 MB bias DMA per
            # (head, k-tile); both q-block OT accumulators stay live in PSUM
            # (2 x 2 banks) next to the double-buffered S tiles (2 x 2).
            # The softmax epilogue runs on DVE + GpSimd only (reciprocal +
            # partition_broadcast), so PE rolls straight into the next head
            # with no >3.4us idle gap (which would re-throttle the HAM
            # clock to 1.2 GHz).
            QB = 1024
            NQB = SEQ // QB
            with tc.tile_pool(name="otpsum", bufs=2, space="PSUM") as otpool, \
                 tc.tile_pool(name="spsum", bufs=2, space="PSUM") as spool, \
                 tc.tile_pool(name="biasp", bufs=7) as biaspool, \
                 tc.tile_pool(name="sbp", bufs=4) as sbpool, \
                 tc.tile_pool(name="ep", bufs=6) as epool:
                for h in range(HPC):
                    hsl = slice(h * DH, (h + 1) * DH)
                    OTs = [otpool.tile([DH + 1, QB], F32, name=f"OT{h}_{qb}",
                                       tag="ot")
                           for qb in range(NQB)]
                    for kt in range(NKT):
                        bias_sb = biaspool.tile([P, SEQ], F32)
                        # spread bias transfers over three DMA paths (two
                        # HWDGE rings + SWDGE) so they overlap instead of
                        # serializing on one FIFO
                        dma_eng = (nc.sync, nc.scalar, nc.gpsimd)[kt % 3]
                        dma_eng.dma_start(bias_sb, biasT[h, bass.ts(kt, P), :])
                        for qb in range(NQB):
                            q0 = qb * QB
                            S = spool.tile([P, QB], F32, tag="s")
                            if QK_DT == "bf16split":
                                # group same-stationary MMs so the PE keeps
                                # one LDWEIGHTS per weight set (background
                                # double-buffered)
                                for j in range(2):
                                    nc.tensor.matmul(
                                        S[:, bass.ts(j, 512)],
                                        KhKl[h][:, bass.ts(kt, P)],
                                        QhQh[h][:, bass.ds(q0 + j * 512, 512)],
                                        start=True, stop=False)
                                for j in range(2):
                                    nc.tensor.matmul(
                                        S[:, bass.ts(j, 512)],
                                        KhKl[h][0:DH, bass.ts(kt, P)],
                                        Qlo[h][:, bass.ds(q0 + j * 512, 512)],
                                        start=False, stop=True)
                            else:
                                for j in range(2):
                                    nc.tensor.matmul(
                                        S[:, bass.ts(j, 512)],
                                        KT_sb[hsl, bass.ts(kt, P)],
                                        QT_sb[hsl, bass.ds(q0 + j * 512, 512)],
                                        start=True, stop=True)
                            SB = sbpool.tile([P, QB], F32, tag="SB")
                            nc.vector.tensor_add(SB, S, bias_sb[:, bass.ds(q0, QB)])
                            E = epool.tile([P, QB], _dt(AV_DT))
                            nc.scalar.activation(E, SB, mybir.ActivationFunctionType.Exp)
                            for j in range(2):
                                nc.tensor.matmul(
                                    OTs[qb][:, bass.ts(j, 512)],
                                    V_sb[:, h, kt, :],
                                    E[:, bass.ts(j, 512)],
                                    start=(kt == 0), stop=(kt == NKT - 1))
                    # gate (unnormalized) and stash the exp-sum row; the
                    # softmax division happens on the host
                    for qb in range(NQB):
                        q0 = qb * QB
                        OT = OTs[qb]
                        # on the last head the exp-sum copy rides ACT so
                        # the DVE epilogue chain stays under the ~3.4us HAM
                        # re-throttle window before the output projections
                        if h == HPC - 1:
                            nc.scalar.copy(rs_sb[:, h, bass.ds(q0, QB)],
                                           OT[DH:DH + 1, :])
                        else:
                            nc.vector.tensor_copy(rs_sb[:, h, bass.ds(q0, QB)],
                                                  OT[DH:DH + 1, :])
                        nc.vector.tensor_mul(OG_sb[:, h, bass.ds(q0, QB)],
                                             GT_sb[hsl, bass.ds(q0, QB)],
                                             OT[0:DH, :])


                # ---- stage D: per-head output projections (partials) ----
                # inside the attention pool scope, with PSUM riding the
                # S-pool slots: no pool-close barrier, so head 0's finals
                # (ready since mid-kernel) start the moment an S slot frees
                # after the last exp, covering head 1's epilogue on DVE and
                # keeping the PE clock warm into the tail.
                for h in range(HPC):
                    for tt in range(SEQ // P):
                        ps = spool.tile([P, CQ], F32, tag="s", name="fin_ps")
                        nc.tensor.matmul(ps, OG_sb[:, h, bass.ts(tt, P)],
                                         wo_sb[:, h, :], start=True, stop=True)
                        o_sb = sbpool.tile([P, CQ], F32, tag="SB",
                                           name="fin_osb")
                        nc.vector.tensor_copy(o_sb, ps)
                        eng = nc.sync if tt % 2 == 0 else nc.scalar
                        eng.dma_start(outs_d[h][bass.ts(tt, P), :], o_sb)

            nc.sync.dma_start(rs_d, rs_sb)

    nc.compile()
    return nc


_NC = None
_NC_LOCK = threading.Lock()


def _get_nc():
    global _NC
    with _NC_LOCK:
        if _NC is None:
            _NC = build_nc()
        return _NC


def make_in_maps(q_x, kv_x, bias, w_q, w_k, w_v, w_g, b_g, w_o, b_o):
    del b_o  # added on the host after the gather
    q_x = np.asarray(q_x, dtype=np.float32)
    kv_x = np.asarray(kv_x, dtype=np.float32)
    bias = np.asarray(bias, dtype=np.float32)
    w_q = np.asarray(w_q, dtype=np.float32) * np.float32(0.125)  # fold 1/sqrt(64)
    w_k = np.asarray(w_k, dtype=np.float32)
    w_v = np.asarray(w_v, dtype=np.float32)
    w_g = np.asarray(w_g, dtype=np.float32)
    b_g = np.asarray(b_g, dtype=np.float32)
    w_o = np.asarray(w_o, dtype=np.float32)

    in_maps = []
    for c in range(N_CORES):
        b = c // (N_CORES // B)
        h0 = HPC * (c % (N_CORES // B))
        cols = slice(h0 * DH, (h0 + HPC) * DH)
        in_maps.append({
            "qxT": np.ascontiguousarray(q_x[b].T),
            "kvxT": np.ascontiguousarray(kv_x[b].T),
            "biasT": np.ascontiguousarray(bias[b, h0:h0 + HPC].swapaxes(1, 2)),
            "wq": np.ascontiguousarray(w_q[:, cols]),
            "wk": np.ascontiguousarray(w_k[:, cols]),
            "wv": np.ascontiguousarray(w_v[:, cols]),
            "wg": np.ascontiguousarray(w_g[:, cols]),
            "bg": np.ascontiguousarray(b_g[cols].reshape(HPC * DH, 1)),
            "wo": np.ascontiguousarray(w_o[cols, :]),
        })
    return in_maps


def gather_output(results, b_o):
    full = np.zeros((B, SEQ, CQ), dtype=np.float32)
    for c in range(N_CORES):
        b = c // (N_CORES // B)
        rs = results[c]["rs"][0]
        for h in range(HPC):
            full[b] += results[c][f"out{h}"] / rs[h][:, None]
    full += np.asarray(b_o, dtype=np.float32)
    return full


def kernel(**inputs):
    nc = _get_nc()
    in_maps = make_in_maps(**inputs)
    res = run_bass_kernel_spmd(nc, in_maps, core_ids=list(range(N_CORES)))
    return gather_output(res.results, inputs["b_o"])

